# revision 16
# baseline (speedup 1.0000x reference)
"""Multi-head attention layer (N=4, L=S=2048, D=1024, H=16) on 8 TRN2 NeuronCores.

Sharding: 8 cores = 4 batches x 2 query-halves (heads kept local, so no
collectives: each core computes Q projection for its 1024 query rows, K/V
projections for the full 2048 keys of its batch, all 16 heads of attention,
and the output projection for its rows). Host shards/gathers.

Per-core data layout (host-prepared, bf16):
  xq [128, 8, 1024]  xq[p,t,l] = queries[n, l0+l, t*128+p]   (transposed)
  xk/xv [128, 8, 2048]  keys[n].T / values[n].T, same packing
  wq/wk/wv/wo [128, 8, 1024]  w[p,t,d] = W[t*128+p, d]
  bq/bk [128, 8] f32; bv [64, 16] f32; bo [128, 1024] f32 (pre-broadcast)
  out [1024, 1024] f32 (natural layout)

Pipeline notes:
- Attention processes HEAD PAIRS (2dt, 2dt+1): the two heads' K/Q live at
  partition bases 0/64 of d-tile dt, so their K=64 scores matmuls target
  different PE row groups and run concurrently (tile_position auto-derived).
- exp is batched over [128,1024] PSUM pairs (ScalarE runs 1 elem/lane/cycle
  at 1.2 GHz + 352-cycle per-instruction overhead — it is the attention
  pacer, so attention-phase PE work must stay below it).
- K-projection d-tiles and Q(lb) groups are emitted just-in-time as PE
  filler inside earlier attention units, keeping the PE dense so HAM never
  re-throttles it to 1.2 GHz; only V + first K/Q tiles run up front.
- Softmax denominator comes free from a ones-column appended to V (PV
  matmul has 65 output rows; row 64 = sum of exp). attn rows sum to 1, so
  V's bias is added after normalization (P@(V+bv) = P@V + bv).
- PV accumulators are copied PSUM->SBUF immediately so banks recycle;
  normalization (broadcast reciprocal multiply) runs off-critical-path.
"""

import numpy as np
import ml_dtypes

import concourse.bass as bass
import concourse.mybir as mybir
import concourse.tile as tile
from concourse import bacc
from concourse.bass_utils import run_bass_kernel_spmd

BF16 = mybir.dt.bfloat16
F32 = mybir.dt.float32
ALU = mybir.AluOpType
ACTF = mybir.ActivationFunctionType

N, L, S, D, H, E = 4, 2048, 2048, 1024, 16, 64
LQ = 1024
N_CORES = 8

_nc_cache = None
last_results = None


def _build():
    nc = bacc.Bacc(None, target_bir_lowering=False)

    xq = nc.declare_dram_parameter("xq", [128, 8, LQ], BF16, isOutput=False)
    xk = nc.declare_dram_parameter("xk", [128, 8, S], BF16, isOutput=False)
    xv = nc.declare_dram_parameter("xv", [128, 8, S], BF16, isOutput=False)
    wq = nc.declare_dram_parameter("wq", [128, 8, D], BF16, isOutput=False)
    wk = nc.declare_dram_parameter("wk", [128, 8, D], BF16, isOutput=False)
    wv = nc.declare_dram_parameter("wv", [128, 8, D], BF16, isOutput=False)
    wo = nc.declare_dram_parameter("wo", [128, 8, D], BF16, isOutput=False)
    bq = nc.declare_dram_parameter("bq", [128, 8], F32, isOutput=False)
    bk = nc.declare_dram_parameter("bk", [128, 8], F32, isOutput=False)
    bv = nc.declare_dram_parameter("bv", [64, 16], F32, isOutput=False)
    bo = nc.declare_dram_parameter("bo", [128, D], F32, isOutput=False)
    out = nc.declare_dram_parameter("out", [LQ, D], F32, isOutput=True)

    with tile.TileContext(nc) as tc:
        with tc.tile_pool(name="const", bufs=1) as cpool, \
             tc.tile_pool(name="pers", bufs=1) as ppool, \
             tc.tile_pool(name="stage", bufs=2) as spool, \
             tc.tile_pool(name="work", bufs=2) as wpool, \
             tc.tile_pool(name="expp", bufs=4) as epool, \
             tc.tile_pool(name="psum", bufs=2, space="PSUM") as psum:

            wq_t = cpool.tile([128, 8, D], BF16, tag="w_a")
            wk_t = cpool.tile([128, 8, D], BF16, tag="w_b")
            wv_t = cpool.tile([128, 8, D], BF16, tag="w_c")
            nc.sync.dma_start(wv_t[:], wv[:])
            bq_t = cpool.tile([128, 8], F32, tag="bq")
            bk_t = cpool.tile([128, 8], F32, tag="bk")
            bv_t = cpool.tile([64, 16], F32, tag="bv")
            bo_t = cpool.tile([128, D], F32, tag="bo")
            nc.sync.dma_start(bq_t[:], bq[:])
            nc.sync.dma_start(bk_t[:], bk[:])
            nc.sync.dma_start(bv_t[:], bv[:])
            nc.sync.dma_start(bo_t[:], bo[:])

            qT = ppool.tile([128, 8, LQ], BF16, tag="qT")
            kT = ppool.tile([128, 8, S], BF16, tag="kT")
            vaug = ppool.tile([128, 16, 16 * 65], BF16, tag="vaug")
            oT = ppool.tile([128, 8, LQ], BF16, tag="oT")

            for st in range(16):
                v3 = vaug[:, st].rearrange("p (h e) -> p h e", e=65)
                nc.vector.memset(v3[:, :, 64:65], 1.0)

            # one PSUM projection group: 8 accumulating matmuls + epilogue.
            # mm512 is shared by projections, PV accumulators and the output
            # projection (4 banks); sc2 (scores pairs) has the other 4.
            def proj_group(w_t, sg_t, dt, dst, bias):
                ps = psum.tile([128, 512], F32, tag="mm512", bufs=2)
                for ct in range(8):
                    nc.tensor.matmul(ps[:], w_t[:, ct, dt * 128:(dt + 1) * 128],
                                     sg_t[:, ct, :], start=(ct == 0),
                                     stop=(ct == 7))
                nc.vector.tensor_scalar_add(dst, ps[:], bias)

            def q_stage(lb):
                sg = spool.tile([128, 8, 512], BF16, tag="stage")
                nc.sync.dma_start(sg[:], xq[:, :, lb * 512:(lb + 1) * 512])
                return sg

            def k_stage(sb):
                sg = spool.tile([128, 8, 512], BF16, tag="stage")
                nc.sync.dma_start(sg[:], xk[:, :, sb * 512:(sb + 1) * 512])
                return sg

            def q_item(dt, lb):
                def compute(sg):
                    proj_group(wq_t, sg, dt,
                               qT[:, dt, lb * 512:(lb + 1) * 512],
                               bq_t[:, dt:dt + 1])
                return (lambda lb=lb: q_stage(lb)), compute

            def k_item(dt, sb):
                def compute(sg):
                    proj_group(wk_t, sg, dt,
                               kT[:, dt, sb * 512:(sb + 1) * 512],
                               bk_t[:, dt:dt + 1])
                return (lambda sb=sb: k_stage(sb)), compute

            def q_group(dt, lb):
                dma, compute = q_item(dt, lb)
                compute(dma())

            def k_group(dt, sb):
                dma, compute = k_item(dt, sb)
                compute(dma())

            def v_proj_group(sg_t, stl, st, db):
                ps = psum.tile([128, 512], F32, tag="mm512", bufs=2)
                for ct in range(8):
                    nc.tensor.matmul(ps[:], sg_t[:, ct, stl * 128:(stl + 1) * 128],
                                     wv_t[:, ct, db * 512:(db + 1) * 512],
                                     start=(ct == 0), stop=(ct == 7))
                v3 = vaug[:, st].rearrange("p (h e) -> p h e", e=65)
                nc.vector.tensor_copy(
                    v3[:, db * 8:(db + 1) * 8, 0:64],
                    ps[:].rearrange("p (h e) -> p h e", e=64))

            def o_proj_group(lt, db):
                ps = psum.tile([128, 512], F32, tag="mm512", bufs=2)
                for ct in range(8):
                    nc.tensor.matmul(ps[:], oT[:, ct, lt * 128:(lt + 1) * 128],
                                     wo_t[:, ct, db * 512:(db + 1) * 512],
                                     start=(ct == 0), stop=(ct == 7))
                ob = wpool.tile([128, 512], F32, tag="outsb")
                nc.vector.tensor_add(ob[:], ps[:],
                                     bo_t[:, db * 512:(db + 1) * 512])
                nc.sync.dma_start(
                    out[lt * 128:(lt + 1) * 128, db * 512:(db + 1) * 512], ob[:])

            def normalize(cp, h, lb):
                # cp: [65, 512] f32 SBUF; row 64 = softmax denominator
                den0 = wpool.tile([1, 512], F32, tag="rec0")
                nc.sync.dma_start(den0[0:1, :], cp[64:65, :])
                denb = wpool.tile([64, 512], F32, tag="recb")
                nc.gpsimd.partition_broadcast(denb[:], den0[0:1, :])
                recb = wpool.tile([64, 512], F32, tag="recf")
                nc.vector.reciprocal_approx_fast(recb[:], denb[:])
                dt = h // 2
                if h % 2 == 0:
                    dst = oT[0:64, dt, lb * 512:(lb + 1) * 512]
                    nc.vector.tensor_tensor(dst, cp[0:64, :], recb[:], ALU.mult)
                    nc.vector.tensor_scalar_add(dst, dst, bv_t[:, h:h + 1])
                else:
                    tmp = wpool.tile([64, 512], BF16, tag="otmp")
                    nc.vector.tensor_tensor(tmp[:], cp[0:64, :], recb[:],
                                            ALU.mult)
                    nc.vector.tensor_scalar_add(tmp[:], tmp[:], bv_t[:, h:h + 1])
                    nc.sync.dma_start(
                        oT[64:128, dt, lb * 512:(lb + 1) * 512], tmp[:])

            pending = []

            def pump(feed):
                try:
                    dma, compute = next(feed)
                except StopIteration:
                    if pending:
                        pending.pop(0)()
                    return
                sg = dma() if dma else None
                pending.append(lambda c=compute, s=sg: c(s))
                if len(pending) > 2:
                    pending.pop(0)()

            def flush():
                while pending:
                    pending.pop(0)()

            def attention_pair(dt, lb, feed):
                he, ho = 2 * dt, 2 * dt + 1
                qe = qT[0:64, dt, lb * 512:(lb + 1) * 512]
                qo = qT[64:128, dt, lb * 512:(lb + 1) * 512]
                pe = psum.tile([128, 512], F32, tag="pepo", bufs=2)
                po = psum.tile([128, 512], F32, tag="pepo", bufs=2)
                for st in range(16):
                    ps2 = psum.tile([128, 1024], F32, tag="sc2", bufs=2)
                    # concurrent pair: row groups (0,0) and (64,0)
                    nc.tensor.matmul(ps2[:, 0:512],
                                     kT[0:64, dt, st * 128:(st + 1) * 128],
                                     qe, start=True, stop=True)
                    nc.tensor.matmul(ps2[:, 512:1024],
                                     kT[64:128, dt, st * 128:(st + 1) * 128],
                                     qo, start=True, stop=True)
                    ep = epool.tile([128, 1024], BF16, tag="ep")
                    nc.scalar.activation(ep[:], ps2[:], ACTF.Exp, scale=0.125)
                    if st % 2 == 0:
                        pump(feed)
                    nc.tensor.matmul(pe[0:65, :],
                                     vaug[:, st, he * 65:(he + 1) * 65],
                                     ep[:, 0:512],
                                     start=(st == 0), stop=(st == 15))
                    nc.tensor.matmul(po[0:65, :],
                                     vaug[:, st, ho * 65:(ho + 1) * 65],
                                     ep[:, 512:1024],
                                     start=(st == 0), stop=(st == 15))
                # free the accumulator banks right away
                cpe = wpool.tile([65, 512], F32, tag="cpe")
                nc.vector.tensor_copy(cpe[:], pe[0:65, :])
                cpo = wpool.tile([65, 512], F32, tag="cpo")
                nc.vector.tensor_copy(cpo[:], po[0:65, :])
                normalize(cpe, he, lb)
                normalize(cpo, ho, lb)

            # ---- up-front: V (all), K(dt=0), Q(dt=0, lb=0) ----
            for sb in range(4):
                sg = spool.tile([128, 8, 512], BF16, tag="stage")
                nc.sync.dma_start(sg[:], xv[:, :, sb * 512:(sb + 1) * 512])
                for stl in range(4):
                    for db in range(2):
                        v_proj_group(sg, stl, sb * 4 + stl, db)
            nc.sync.dma_start(wk_t[:], wk[:])
            nc.sync.dma_start(wq_t[:], wq[:])
            for sb in range(4):
                k_group(0, sb)
            q_group(0, 0)

            # ---- attention sweeps; projections ride along as PE filler ----
            def lb0_items():
                for d in range(1, 8):
                    for sb in range(4):
                        yield k_item(d, sb)
                    yield q_item(d, 0)
                    yield q_item(d - 1, 1)
                yield q_item(7, 1)

            def lb1_items():
                for lt in range(4):
                    for db in range(2):
                        yield (None,
                               lambda sg, lt=lt, db=db: o_proj_group(lt, db))

            feed = lb0_items()
            for dt in range(8):
                attention_pair(dt, 0, feed)
            flush()

            # wo reuses wq's slot: every q_group was emitted in the lb=0 sweep
            wo_t = cpool.tile([128, 8, D], BF16, tag="w_a")
            nc.sync.dma_start(wo_t[:], wo[:])

            feed = lb1_items()
            for dt in range(8):
                attention_pair(dt, 1, feed)
            flush()

            for lt in range(4, 8):
                for db in range(2):
                    o_proj_group(lt, db)

    nc.compile()
    return nc


def _pack_kxm(w):
    k, m = w.shape
    return np.ascontiguousarray(
        w.reshape(k // 128, 128, m).transpose(1, 0, 2)).astype(ml_dtypes.bfloat16)


def kernel(queries, keys, values, Wq, bq, Wk, bk, Wv, bv, Wo, bo):
    global _nc_cache, last_results
    queries = np.asarray(queries, dtype=np.float32)
    keys = np.asarray(keys, dtype=np.float32)
    values = np.asarray(values, dtype=np.float32)

    if _nc_cache is None:
        _nc_cache = _build()
    nc = _nc_cache

    w_packed = {
        "wq": _pack_kxm(np.asarray(Wq, np.float32)),
        "wk": _pack_kxm(np.asarray(Wk, np.float32)),
        "wv": _pack_kxm(np.asarray(Wv, np.float32)),
        "wo": _pack_kxm(np.asarray(Wo, np.float32)),
        "bq": np.ascontiguousarray(np.asarray(bq, np.float32).reshape(8, 128).T),
        "bk": np.ascontiguousarray(np.asarray(bk, np.float32).reshape(8, 128).T),
        "bv": np.ascontiguousarray(np.asarray(bv, np.float32).reshape(16, 64).T),
        "bo": np.ascontiguousarray(
            np.broadcast_to(np.asarray(bo, np.float32), (128, D))),
    }

    in_maps = []
    for c in range(N_CORES):
        n, half = c // 2, c % 2
        m = dict(w_packed)
        m["xq"] = _pack_kxm(
            np.ascontiguousarray(queries[n, half * LQ:(half + 1) * LQ, :].T))
        m["xk"] = _pack_kxm(np.ascontiguousarray(keys[n].T))
        m["xv"] = _pack_kxm(np.ascontiguousarray(values[n].T))
        in_maps.append(m)

    last_results = run_bass_kernel_spmd(nc, in_maps, list(range(N_CORES)))

    full = np.empty((N, L, D), np.float32)
    for c in range(N_CORES):
        n, half = c // 2, c % 2
        full[n, half * LQ:(half + 1) * LQ, :] = last_results.results[c]["out"]
    return full


# revision 18
# speedup vs baseline: 1.1965x; 1.1965x over previous
"""Multi-head attention layer (N=4, L=S=2048, D=1024, H=16) on 8 TRN2 NeuronCores.

Sharding: 8 cores = 4 batches x 2 query-halves (heads kept local, so no
collectives: each core computes Q projection for its 1024 query rows, K/V
projections for the full 2048 keys of its batch, all 16 heads of attention,
and the output projection for its rows). Host shards/gathers.

Per-core data layout (host-prepared, bf16):
  xq [128, 8, 1024]  xq[p,t,l] = queries[n, l0+l, t*128+p]   (transposed)
  xk/xv [128, 8, 2048]  keys[n].T / values[n].T, same packing
  wq/wk/wv/wo [128, 8, 1024]  w[p,t,d] = W[t*128+p, d]
  bq/bk [128, 8] f32; bv [64, 16] f32; bo [128, 1024] f32 (pre-broadcast)
  out [1024, 1024] f32 (natural layout)

Pipeline notes:
- Attention processes HEAD PAIRS (2dt, 2dt+1): the two heads' K/Q live at
  partition bases 0/64 of d-tile dt, so their K=64 scores matmuls target
  different PE row groups and run concurrently (tile_position auto-derived).
- exp is batched over [128,1024] PSUM pairs (ScalarE runs 1 elem/lane/cycle
  at 1.2 GHz + 352-cycle per-instruction overhead — it is the attention
  pacer, so attention-phase PE work must stay below it).
- K-projection d-tiles and Q(lb) groups are emitted just-in-time as PE
  filler inside earlier attention units, keeping the PE dense so HAM never
  re-throttles it to 1.2 GHz; only V + first K/Q tiles run up front.
- Softmax denominator comes free from a ones-column appended to V (PV
  matmul has 65 output rows; row 64 = sum of exp). attn rows sum to 1, so
  V's bias is added after normalization (P@(V+bv) = P@V + bv).
- PV accumulators are copied PSUM->SBUF immediately so banks recycle;
  normalization (broadcast reciprocal multiply) runs off-critical-path.
"""

import numpy as np
import ml_dtypes

import concourse.bass as bass
import concourse.mybir as mybir
import concourse.tile as tile
from concourse import bacc
from concourse.bass_utils import run_bass_kernel_spmd

BF16 = mybir.dt.bfloat16
F32 = mybir.dt.float32
ALU = mybir.AluOpType
ACTF = mybir.ActivationFunctionType

N, L, S, D, H, E = 4, 2048, 2048, 1024, 16, 64
LQ = 1024
N_CORES = 8

_nc_cache = None
last_results = None


def _build():
    nc = bacc.Bacc(None, target_bir_lowering=False)

    xq = nc.declare_dram_parameter("xq", [128, 8, LQ], BF16, isOutput=False)
    xk = nc.declare_dram_parameter("xk", [128, 8, S], BF16, isOutput=False)
    xv = nc.declare_dram_parameter("xv", [128, 8, S], BF16, isOutput=False)
    wq = nc.declare_dram_parameter("wq", [128, 8, D], BF16, isOutput=False)
    wk = nc.declare_dram_parameter("wk", [128, 8, D], BF16, isOutput=False)
    wv = nc.declare_dram_parameter("wv", [128, 8, D], BF16, isOutput=False)
    wo = nc.declare_dram_parameter("wo", [128, 8, D], BF16, isOutput=False)
    bq = nc.declare_dram_parameter("bq", [128, 8], F32, isOutput=False)
    bk = nc.declare_dram_parameter("bk", [128, 8], F32, isOutput=False)
    bv = nc.declare_dram_parameter("bv", [64, 16], F32, isOutput=False)
    bo = nc.declare_dram_parameter("bo", [128, D], F32, isOutput=False)
    out = nc.declare_dram_parameter("out", [LQ, D], F32, isOutput=True)

    with tile.TileContext(nc) as tc:
        with tc.tile_pool(name="const", bufs=1) as cpool, \
             tc.tile_pool(name="pers", bufs=1) as ppool, \
             tc.tile_pool(name="stage", bufs=3) as spool, \
             tc.tile_pool(name="work", bufs=2) as wpool, \
             tc.tile_pool(name="expp", bufs=4) as epool, \
             tc.tile_pool(name="psum", bufs=2, space="PSUM") as psum:

            wq_t = cpool.tile([128, 8, D], BF16, tag="w_a")
            wk_t = cpool.tile([128, 8, D], BF16, tag="w_b")
            wv_t = cpool.tile([128, 8, D], BF16, tag="w_c")
            nc.sync.dma_start(wv_t[:], wv[:])
            bq_t = cpool.tile([128, 8], F32, tag="bq")
            bk_t = cpool.tile([128, 8], F32, tag="bk")
            bv_t = cpool.tile([64, 16], F32, tag="bv")
            bo_t = cpool.tile([128, D], F32, tag="bo")
            nc.sync.dma_start(bq_t[:], bq[:])
            nc.sync.dma_start(bk_t[:], bk[:])
            nc.sync.dma_start(bv_t[:], bv[:])
            nc.sync.dma_start(bo_t[:], bo[:])

            qT = ppool.tile([128, 8, LQ], BF16, tag="qT")
            kT = ppool.tile([128, 8, S], BF16, tag="kT")
            vaug = ppool.tile([128, 16, 16 * 65], BF16, tag="vaug")
            oT = ppool.tile([128, 8, LQ], BF16, tag="oT")

            for st in range(16):
                v3 = vaug[:, st].rearrange("p (h e) -> p h e", e=65)
                nc.vector.memset(v3[:, :, 64:65], 1.0)

            # one PSUM projection group: 8 accumulating matmuls + epilogue.
            # mm512 is shared by projections, PV accumulators and the output
            # projection (4 banks); sc2 (scores pairs) has the other 4.
            def proj_group(w_t, sg_t, dt, dst, bias):
                ps = psum.tile([128, 512], F32, tag="mm512", bufs=2)
                for ct in range(8):
                    nc.tensor.matmul(ps[:], w_t[:, ct, dt * 128:(dt + 1) * 128],
                                     sg_t[:, ct, :], start=(ct == 0),
                                     stop=(ct == 7))
                nc.vector.tensor_scalar_add(dst, ps[:], bias)

            def q_stage(lb):
                sg = spool.tile([128, 8, 512], BF16, tag="stage")
                nc.sync.dma_start(sg[:], xq[:, :, lb * 512:(lb + 1) * 512])
                return sg

            def k_stage(sb):
                sg = spool.tile([128, 8, 512], BF16, tag="stage")
                nc.sync.dma_start(sg[:], xk[:, :, sb * 512:(sb + 1) * 512])
                return sg

            def q_item(dt, lb):
                def compute(sg):
                    proj_group(wq_t, sg, dt,
                               qT[:, dt, lb * 512:(lb + 1) * 512],
                               bq_t[:, dt:dt + 1])
                return (lambda lb=lb: q_stage(lb)), compute

            def k_item(dt, sb):
                def compute(sg):
                    proj_group(wk_t, sg, dt,
                               kT[:, dt, sb * 512:(sb + 1) * 512],
                               bk_t[:, dt:dt + 1])
                return (lambda sb=sb: k_stage(sb)), compute

            def q_group(dt, lb):
                dma, compute = q_item(dt, lb)
                compute(dma())

            def k_group(dt, sb):
                dma, compute = k_item(dt, sb)
                compute(dma())

            def v_proj_group(sg_t, stl, st, db):
                ps = psum.tile([128, 512], F32, tag="mm512", bufs=2)
                for ct in range(8):
                    nc.tensor.matmul(ps[:], sg_t[:, ct, stl * 128:(stl + 1) * 128],
                                     wv_t[:, ct, db * 512:(db + 1) * 512],
                                     start=(ct == 0), stop=(ct == 7))
                v3 = vaug[:, st].rearrange("p (h e) -> p h e", e=65)
                nc.vector.tensor_copy(
                    v3[:, db * 8:(db + 1) * 8, 0:64],
                    ps[:].rearrange("p (h e) -> p h e", e=64))

            def o_proj_group(lt, db):
                ps = psum.tile([128, 512], F32, tag="mm512", bufs=2)
                for ct in range(8):
                    nc.tensor.matmul(ps[:], oT[:, ct, lt * 128:(lt + 1) * 128],
                                     wo_t[:, ct, db * 512:(db + 1) * 512],
                                     start=(ct == 0), stop=(ct == 7))
                ob = wpool.tile([128, 512], F32, tag="outsb")
                nc.vector.tensor_add(ob[:], ps[:],
                                     bo_t[:, db * 512:(db + 1) * 512])
                nc.sync.dma_start(
                    out[lt * 128:(lt + 1) * 128, db * 512:(db + 1) * 512], ob[:])

            def normalize(cp, h, lb):
                # cp: [65, 512] f32 SBUF; row 64 = softmax denominator
                den0 = wpool.tile([1, 512], F32, tag="rec0")
                nc.sync.dma_start(den0[0:1, :], cp[64:65, :])
                denb = wpool.tile([64, 512], F32, tag="recb")
                nc.gpsimd.partition_broadcast(denb[:], den0[0:1, :])
                recb = wpool.tile([64, 512], F32, tag="recf")
                nc.vector.reciprocal_approx_fast(recb[:], denb[:])
                dt = h // 2
                if h % 2 == 0:
                    dst = oT[0:64, dt, lb * 512:(lb + 1) * 512]
                    nc.vector.tensor_tensor(dst, cp[0:64, :], recb[:], ALU.mult)
                    nc.vector.tensor_scalar_add(dst, dst, bv_t[:, h:h + 1])
                else:
                    tmp = wpool.tile([64, 512], BF16, tag="otmp")
                    nc.vector.tensor_tensor(tmp[:], cp[0:64, :], recb[:],
                                            ALU.mult)
                    nc.vector.tensor_scalar_add(tmp[:], tmp[:], bv_t[:, h:h + 1])
                    nc.sync.dma_start(
                        oT[64:128, dt, lb * 512:(lb + 1) * 512], tmp[:])

            pending = []

            def pump(feed):
                try:
                    dma, compute = next(feed)
                except StopIteration:
                    if pending:
                        pending.pop(0)()
                    return
                sg = dma() if dma else None
                pending.append(lambda c=compute, s=sg: c(s))
                if len(pending) > 2:
                    pending.pop(0)()

            def flush():
                while pending:
                    pending.pop(0)()

            def attention_pair(dt, lb, feed, pop_sts=(2, 5, 8, 11, 14)):
                pump(feed)
                he, ho = 2 * dt, 2 * dt + 1
                qe = qT[0:64, dt, lb * 512:(lb + 1) * 512]
                qo = qT[64:128, dt, lb * 512:(lb + 1) * 512]
                pe = psum.tile([128, 512], F32, tag="pepo", bufs=2)
                po = psum.tile([128, 512], F32, tag="pepo", bufs=2)
                for st in range(16):
                    ps2 = psum.tile([128, 1024], F32, tag="sc2", bufs=2)
                    # concurrent pair: row groups (0,0) and (64,0)
                    nc.tensor.matmul(ps2[:, 0:512],
                                     kT[0:64, dt, st * 128:(st + 1) * 128],
                                     qe, start=True, stop=True)
                    nc.tensor.matmul(ps2[:, 512:1024],
                                     kT[64:128, dt, st * 128:(st + 1) * 128],
                                     qo, start=True, stop=True)
                    ep = epool.tile([128, 1024], BF16, tag="ep")
                    nc.scalar.activation(ep[:], ps2[:], ACTF.Exp, scale=0.125)
                    if st in pop_sts:
                        pump(feed)
                    nc.tensor.matmul(pe[0:65, :],
                                     vaug[:, st, he * 65:(he + 1) * 65],
                                     ep[:, 0:512],
                                     start=(st == 0), stop=(st == 15))
                    nc.tensor.matmul(po[0:65, :],
                                     vaug[:, st, ho * 65:(ho + 1) * 65],
                                     ep[:, 512:1024],
                                     start=(st == 0), stop=(st == 15))
                # boundary flush: every pending projection must be emitted
                # before the next unit reads its output (Tile deps follow
                # emission order)
                while pending:
                    pending.pop(0)()
                cpe = wpool.tile([65, 512], F32, tag="cpe")
                nc.vector.tensor_copy(cpe[:], pe[0:65, :])
                cpo = wpool.tile([65, 512], F32, tag="cpo")
                nc.vector.tensor_copy(cpo[:], po[0:65, :])
                normalize(cpe, he, lb)
                normalize(cpo, ho, lb)

            # ---- up-front: V (all), K(dt=0), Q(dt=0, lb=0) ----
            for sb in range(4):
                sg = spool.tile([128, 8, 512], BF16, tag="stage")
                nc.sync.dma_start(sg[:], xv[:, :, sb * 512:(sb + 1) * 512])
                for stl in range(4):
                    for db in range(2):
                        v_proj_group(sg, stl, sb * 4 + stl, db)
            nc.sync.dma_start(wk_t[:], wk[:])
            nc.sync.dma_start(wq_t[:], wq[:])
            for sb in range(4):
                k_group(0, sb)
            q_group(0, 0)

            # ---- attention sweeps; projections ride along as PE filler ----
            def lb0_items():
                for d in range(1, 8):
                    yield q_item(d, 0)
                    for sb in range(4):
                        yield k_item(d, sb)
                for d in range(8):
                    yield q_item(d, 1)

            def lb1_items():
                for lt in range(4):
                    for db in range(2):
                        yield (None,
                               lambda sg, lt=lt, db=db: o_proj_group(lt, db))

            feed = lb0_items()
            for dt in range(8):
                attention_pair(dt, 0, feed)
            flush()

            # wo reuses wq's slot: every q_group was emitted in the lb=0 sweep
            wo_t = cpool.tile([128, 8, D], BF16, tag="w_a")
            nc.sync.dma_start(wo_t[:], wo[:])

            feed = lb1_items()
            for dt in range(8):
                attention_pair(dt, 1, feed, pop_sts=(5, 11))
            flush()

            for lt in range(4, 8):
                for db in range(2):
                    o_proj_group(lt, db)

    nc.compile()
    return nc


def _pack_kxm(w):
    k, m = w.shape
    return np.ascontiguousarray(
        w.reshape(k // 128, 128, m).transpose(1, 0, 2)).astype(ml_dtypes.bfloat16)


def kernel(queries, keys, values, Wq, bq, Wk, bk, Wv, bv, Wo, bo):
    global _nc_cache, last_results
    queries = np.asarray(queries, dtype=np.float32)
    keys = np.asarray(keys, dtype=np.float32)
    values = np.asarray(values, dtype=np.float32)

    if _nc_cache is None:
        _nc_cache = _build()
    nc = _nc_cache

    w_packed = {
        "wq": _pack_kxm(np.asarray(Wq, np.float32)),
        "wk": _pack_kxm(np.asarray(Wk, np.float32)),
        "wv": _pack_kxm(np.asarray(Wv, np.float32)),
        "wo": _pack_kxm(np.asarray(Wo, np.float32)),
        "bq": np.ascontiguousarray(np.asarray(bq, np.float32).reshape(8, 128).T),
        "bk": np.ascontiguousarray(np.asarray(bk, np.float32).reshape(8, 128).T),
        "bv": np.ascontiguousarray(np.asarray(bv, np.float32).reshape(16, 64).T),
        "bo": np.ascontiguousarray(
            np.broadcast_to(np.asarray(bo, np.float32), (128, D))),
    }

    in_maps = []
    for c in range(N_CORES):
        n, half = c // 2, c % 2
        m = dict(w_packed)
        m["xq"] = _pack_kxm(
            np.ascontiguousarray(queries[n, half * LQ:(half + 1) * LQ, :].T))
        m["xk"] = _pack_kxm(np.ascontiguousarray(keys[n].T))
        m["xv"] = _pack_kxm(np.ascontiguousarray(values[n].T))
        in_maps.append(m)

    last_results = run_bass_kernel_spmd(nc, in_maps, list(range(N_CORES)))

    full = np.empty((N, L, D), np.float32)
    for c in range(N_CORES):
        n, half = c // 2, c % 2
        full[n, half * LQ:(half + 1) * LQ, :] = last_results.results[c]["out"]
    return full


# revision 20
# speedup vs baseline: 1.2011x; 1.0039x over previous
"""Multi-head attention layer (N=4, L=S=2048, D=1024, H=16) on 8 TRN2 NeuronCores.

Sharding: 8 cores = 4 batches x 2 query-halves (heads kept local, so no
collectives: each core computes Q projection for its 1024 query rows, K/V
projections for the full 2048 keys of its batch, all 16 heads of attention,
and the output projection for its rows). Host shards/gathers.

Per-core data layout (host-prepared, bf16):
  xq [128, 8, 1024]  xq[p,t,l] = queries[n, l0+l, t*128+p]   (transposed)
  xk/xv [128, 8, 2048]  keys[n].T / values[n].T, same packing
  wq/wk/wv/wo [128, 8, 1024]  w[p,t,d] = W[t*128+p, d]
  bq/bk [128, 8] f32; bv [64, 16] f32; bo [128, 1024] f32 (pre-broadcast)
  out [1024, 1024] f32 (natural layout)

Pipeline notes:
- Attention processes HEAD PAIRS (2dt, 2dt+1): the two heads' K/Q live at
  partition bases 0/64 of d-tile dt, so their K=64 scores matmuls target
  different PE row groups and run concurrently (tile_position auto-derived).
- exp is batched over [128,1024] PSUM pairs (ScalarE runs 1 elem/lane/cycle
  at 1.2 GHz + 352-cycle per-instruction overhead — it is the attention
  pacer, so attention-phase PE work must stay below it).
- K-projection d-tiles and Q(lb) groups are emitted just-in-time as PE
  filler inside earlier attention units, keeping the PE dense so HAM never
  re-throttles it to 1.2 GHz; only V + first K/Q tiles run up front.
- Softmax denominator comes free from a ones-column appended to V (PV
  matmul has 65 output rows; row 64 = sum of exp). attn rows sum to 1, so
  V's bias is added after normalization (P@(V+bv) = P@V + bv).
- PV accumulators are copied PSUM->SBUF immediately so banks recycle;
  normalization (broadcast reciprocal multiply) runs off-critical-path.
"""

import numpy as np
import ml_dtypes

import concourse.bass as bass
import concourse.mybir as mybir
import concourse.tile as tile
from concourse import bacc
from concourse.bass_utils import run_bass_kernel_spmd

BF16 = mybir.dt.bfloat16
F32 = mybir.dt.float32
ALU = mybir.AluOpType
ACTF = mybir.ActivationFunctionType

N, L, S, D, H, E = 4, 2048, 2048, 1024, 16, 64
LQ = 1024
N_CORES = 8

_nc_cache = None
last_results = None


def _build():
    nc = bacc.Bacc(None, target_bir_lowering=False)

    xq = nc.declare_dram_parameter("xq", [128, 8, LQ], BF16, isOutput=False)
    xk = nc.declare_dram_parameter("xk", [128, 8, S], BF16, isOutput=False)
    xv = nc.declare_dram_parameter("xv", [128, 8, S], BF16, isOutput=False)
    wq = nc.declare_dram_parameter("wq", [128, 8, D], BF16, isOutput=False)
    wk = nc.declare_dram_parameter("wk", [128, 8, D], BF16, isOutput=False)
    wv = nc.declare_dram_parameter("wv", [128, 8, D], BF16, isOutput=False)
    wo = nc.declare_dram_parameter("wo", [128, 8, D], BF16, isOutput=False)
    bq = nc.declare_dram_parameter("bq", [128, 8], F32, isOutput=False)
    bk = nc.declare_dram_parameter("bk", [128, 8], F32, isOutput=False)
    bv = nc.declare_dram_parameter("bv", [64, 16], F32, isOutput=False)
    bo = nc.declare_dram_parameter("bo", [128, D], F32, isOutput=False)
    out = nc.declare_dram_parameter("out", [LQ, D], F32, isOutput=True)

    with tile.TileContext(nc) as tc:
        with tc.tile_pool(name="const", bufs=1) as cpool, \
             tc.tile_pool(name="pers", bufs=1) as ppool, \
             tc.tile_pool(name="stage", bufs=3) as spool, \
             tc.tile_pool(name="work", bufs=2) as wpool, \
             tc.tile_pool(name="expp", bufs=4) as epool, \
             tc.tile_pool(name="psum", bufs=2, space="PSUM") as psum:

            wq_t = cpool.tile([128, 8, D], BF16, tag="w_a")
            wk_t = cpool.tile([128, 8, D], BF16, tag="w_b")
            wv_t = cpool.tile([128, 8, D], BF16, tag="w_c")
            bq_t = cpool.tile([128, 8], F32, tag="bq")
            bk_t = cpool.tile([128, 8], F32, tag="bk")
            bv_t = cpool.tile([64, 16], F32, tag="bv")
            bo_t = cpool.tile([128, D], F32, tag="bo")
            nc.sync.dma_start(wv_t[:], wv[:])
            nc.sync.dma_start(bv_t[:], bv[:])
            qT = ppool.tile([128, 8, LQ], BF16, tag="qT")
            kT = ppool.tile([128, 8, S], BF16, tag="kT")
            vaug = ppool.tile([128, 16, 16 * 65], BF16, tag="vaug")
            oT = ppool.tile([128, 8, LQ], BF16, tag="oT")

            for st in range(16):
                v3 = vaug[:, st].rearrange("p (h e) -> p h e", e=65)
                nc.vector.memset(v3[:, :, 64:65], 1.0)

            # one PSUM projection group: 8 accumulating matmuls + epilogue.
            # mm512 is shared by projections, PV accumulators and the output
            # projection (4 banks); sc2 (scores pairs) has the other 4.
            def proj_group(w_t, sg_t, dt, dst, bias):
                ps = psum.tile([128, 512], F32, tag="mm512", bufs=2)
                for ct in range(8):
                    nc.tensor.matmul(ps[:], w_t[:, ct, dt * 128:(dt + 1) * 128],
                                     sg_t[:, ct, :], start=(ct == 0),
                                     stop=(ct == 7))
                nc.vector.tensor_scalar_add(dst, ps[:], bias)

            def q_stage(lb):
                sg = spool.tile([128, 8, 512], BF16, tag="stage")
                nc.sync.dma_start(sg[:], xq[:, :, lb * 512:(lb + 1) * 512])
                return sg

            def k_stage(sb):
                sg = spool.tile([128, 8, 512], BF16, tag="stage")
                nc.sync.dma_start(sg[:], xk[:, :, sb * 512:(sb + 1) * 512])
                return sg

            def q_item(dt, lb):
                def compute(sg):
                    proj_group(wq_t, sg, dt,
                               qT[:, dt, lb * 512:(lb + 1) * 512],
                               bq_t[:, dt:dt + 1])
                return (lambda lb=lb: q_stage(lb)), compute

            def k_item(dt, sb):
                def compute(sg):
                    proj_group(wk_t, sg, dt,
                               kT[:, dt, sb * 512:(sb + 1) * 512],
                               bk_t[:, dt:dt + 1])
                return (lambda sb=sb: k_stage(sb)), compute

            def q_group(dt, lb):
                dma, compute = q_item(dt, lb)
                compute(dma())

            def k_group(dt, sb):
                dma, compute = k_item(dt, sb)
                compute(dma())

            def v_proj_group(sg_t, stl, st, db):
                ps = psum.tile([128, 512], F32, tag="mm512", bufs=2)
                for ct in range(8):
                    nc.tensor.matmul(ps[:], sg_t[:, ct, stl * 128:(stl + 1) * 128],
                                     wv_t[:, ct, db * 512:(db + 1) * 512],
                                     start=(ct == 0), stop=(ct == 7))
                v3 = vaug[:, st].rearrange("p (h e) -> p h e", e=65)
                nc.vector.tensor_copy(
                    v3[:, db * 8:(db + 1) * 8, 0:64],
                    ps[:].rearrange("p (h e) -> p h e", e=64))

            def o_proj_group(lt, db):
                ps = psum.tile([128, 512], F32, tag="mm512", bufs=2)
                for ct in range(8):
                    nc.tensor.matmul(ps[:], oT[:, ct, lt * 128:(lt + 1) * 128],
                                     wo_t[:, ct, db * 512:(db + 1) * 512],
                                     start=(ct == 0), stop=(ct == 7))
                ob = wpool.tile([128, 512], F32, tag="outsb")
                nc.vector.tensor_add(ob[:], ps[:],
                                     bo_t[:, db * 512:(db + 1) * 512])
                nc.sync.dma_start(
                    out[lt * 128:(lt + 1) * 128, db * 512:(db + 1) * 512], ob[:])

            def normalize(cp, h, lb):
                # cp: [65, 512] f32 SBUF; row 64 = softmax denominator
                den0 = wpool.tile([1, 512], F32, tag="rec0")
                nc.sync.dma_start(den0[0:1, :], cp[64:65, :])
                denb = wpool.tile([64, 512], F32, tag="recb")
                nc.gpsimd.partition_broadcast(denb[:], den0[0:1, :])
                recb = wpool.tile([64, 512], F32, tag="recf")
                nc.vector.reciprocal_approx_fast(recb[:], denb[:])
                dt = h // 2
                if h % 2 == 0:
                    dst = oT[0:64, dt, lb * 512:(lb + 1) * 512]
                    nc.vector.tensor_tensor(dst, cp[0:64, :], recb[:], ALU.mult)
                    nc.vector.tensor_scalar_add(dst, dst, bv_t[:, h:h + 1])
                else:
                    tmp = wpool.tile([64, 512], BF16, tag="otmp")
                    nc.vector.tensor_tensor(tmp[:], cp[0:64, :], recb[:],
                                            ALU.mult)
                    nc.vector.tensor_scalar_add(tmp[:], tmp[:], bv_t[:, h:h + 1])
                    nc.sync.dma_start(
                        oT[64:128, dt, lb * 512:(lb + 1) * 512], tmp[:])

            pending = []

            def pump(feed):
                try:
                    dma, compute = next(feed)
                except StopIteration:
                    if pending:
                        pending.pop(0)()
                    return
                sg = dma() if dma else None
                pending.append(lambda c=compute, s=sg: c(s))
                if len(pending) > 2:
                    pending.pop(0)()

            def flush():
                while pending:
                    pending.pop(0)()

            def attention_pair(dt, lb, feed, pop_sts=(2, 5, 8, 11, 14)):
                pump(feed)
                he, ho = 2 * dt, 2 * dt + 1
                qe = qT[0:64, dt, lb * 512:(lb + 1) * 512]
                qo = qT[64:128, dt, lb * 512:(lb + 1) * 512]
                pe = psum.tile([128, 512], F32, tag="pepo", bufs=2)
                po = psum.tile([128, 512], F32, tag="pepo", bufs=2)
                for st in range(16):
                    ps2 = psum.tile([128, 1024], F32, tag="sc2", bufs=2)
                    # concurrent pair: row groups (0,0) and (64,0)
                    nc.tensor.matmul(ps2[:, 0:512],
                                     kT[0:64, dt, st * 128:(st + 1) * 128],
                                     qe, start=True, stop=True)
                    nc.tensor.matmul(ps2[:, 512:1024],
                                     kT[64:128, dt, st * 128:(st + 1) * 128],
                                     qo, start=True, stop=True)
                    ep = epool.tile([128, 1024], BF16, tag="ep")
                    nc.scalar.activation(ep[:], ps2[:], ACTF.Exp, scale=0.125)
                    if st in pop_sts:
                        pump(feed)
                    nc.tensor.matmul(pe[0:65, :],
                                     vaug[:, st, he * 65:(he + 1) * 65],
                                     ep[:, 0:512],
                                     start=(st == 0), stop=(st == 15))
                    nc.tensor.matmul(po[0:65, :],
                                     vaug[:, st, ho * 65:(ho + 1) * 65],
                                     ep[:, 512:1024],
                                     start=(st == 0), stop=(st == 15))
                # boundary flush: every pending projection must be emitted
                # before the next unit reads its output (Tile deps follow
                # emission order)
                while pending:
                    pending.pop(0)()
                cpe = wpool.tile([65, 512], F32, tag="cpe")
                nc.vector.tensor_copy(cpe[:], pe[0:65, :])
                cpo = wpool.tile([65, 512], F32, tag="cpo")
                nc.vector.tensor_copy(cpo[:], po[0:65, :])
                normalize(cpe, he, lb)
                normalize(cpo, ho, lb)

            # ---- up-front: V (all), K(dt=0), Q(dt=0, lb=0) ----
            for sb in range(4):
                sg = spool.tile([128, 8, 512], BF16, tag="stage")
                nc.sync.dma_start(sg[:], xv[:, :, sb * 512:(sb + 1) * 512])
                for stl in range(4):
                    for db in range(2):
                        v_proj_group(sg, stl, sb * 4 + stl, db)
            nc.sync.dma_start(wk_t[:], wk[:])
            nc.sync.dma_start(wq_t[:], wq[:])
            nc.sync.dma_start(bq_t[:], bq[:])
            nc.sync.dma_start(bk_t[:], bk[:])
            nc.sync.dma_start(bo_t[:], bo[:])
            for sb in range(4):
                k_group(0, sb)
            q_group(0, 0)

            # ---- attention sweeps; projections ride along as PE filler ----
            def lb0_items():
                for d in range(1, 8):
                    yield q_item(d, 0)
                    for sb in range(4):
                        yield k_item(d, sb)
                for d in range(8):
                    yield q_item(d, 1)

            def lb1_items():
                for lt in range(4):
                    for db in range(2):
                        yield (None,
                               lambda sg, lt=lt, db=db: o_proj_group(lt, db))

            feed = lb0_items()
            for dt in range(8):
                attention_pair(dt, 0, feed)
            flush()

            # wo reuses wq's slot: every q_group was emitted in the lb=0 sweep
            wo_t = cpool.tile([128, 8, D], BF16, tag="w_a")
            nc.sync.dma_start(wo_t[:], wo[:])

            feed = lb1_items()
            for dt in range(8):
                attention_pair(dt, 1, feed, pop_sts=(5, 11))
            flush()

            for lt in range(4, 8):
                for db in range(2):
                    o_proj_group(lt, db)

    nc.compile()
    return nc


def _pack_kxm(w):
    k, m = w.shape
    return np.ascontiguousarray(
        w.reshape(k // 128, 128, m).transpose(1, 0, 2)).astype(ml_dtypes.bfloat16)


def kernel(queries, keys, values, Wq, bq, Wk, bk, Wv, bv, Wo, bo):
    global _nc_cache, last_results
    queries = np.asarray(queries, dtype=np.float32)
    keys = np.asarray(keys, dtype=np.float32)
    values = np.asarray(values, dtype=np.float32)

    if _nc_cache is None:
        _nc_cache = _build()
    nc = _nc_cache

    w_packed = {
        "wq": _pack_kxm(np.asarray(Wq, np.float32)),
        "wk": _pack_kxm(np.asarray(Wk, np.float32)),
        "wv": _pack_kxm(np.asarray(Wv, np.float32)),
        "wo": _pack_kxm(np.asarray(Wo, np.float32)),
        "bq": np.ascontiguousarray(np.asarray(bq, np.float32).reshape(8, 128).T),
        "bk": np.ascontiguousarray(np.asarray(bk, np.float32).reshape(8, 128).T),
        "bv": np.ascontiguousarray(np.asarray(bv, np.float32).reshape(16, 64).T),
        "bo": np.ascontiguousarray(
            np.broadcast_to(np.asarray(bo, np.float32), (128, D))),
    }

    in_maps = []
    for c in range(N_CORES):
        n, half = c // 2, c % 2
        m = dict(w_packed)
        m["xq"] = _pack_kxm(
            np.ascontiguousarray(queries[n, half * LQ:(half + 1) * LQ, :].T))
        m["xk"] = _pack_kxm(np.ascontiguousarray(keys[n].T))
        m["xv"] = _pack_kxm(np.ascontiguousarray(values[n].T))
        in_maps.append(m)

    last_results = run_bass_kernel_spmd(nc, in_maps, list(range(N_CORES)))

    full = np.empty((N, L, D), np.float32)
    for c in range(N_CORES):
        n, half = c // 2, c % 2
        full[n, half * LQ:(half + 1) * LQ, :] = last_results.results[c]["out"]
    return full
